# revision 13
# baseline (speedup 1.0000x reference)
"""Trainium2 Bass kernel for nn_CounterFlowNetwork.

Data-parallel over 8 NeuronCores (batch sharded).  v3: on top of the v2
restructure (bf16 everywhere, DMA-xbar transposed x load, PSUM-resident
accumulators, chunk-pair issue interleaving), ALL K=1 bias-injection
matmuls are gone:

 - The per-plate ascending bias -alpha*b_tr is simply not applied on
   device.  The stored gas state drifts by a host-computable constant
   delta_n = delta_{n-1}(I - alpha W_tr) + alpha b_tr per ascending
   plate; the drift is corrected in the descending sigmoid bias table
   (per sweep/plate/ft), in the head bias (for g_8 and for S, the
   driving-force sum), all folded host-side in float64.
 - Descending sigmoid biases (b_eq + (9-n)c3 + drift correction) ride
   ACT's per-partition bias port with per-ft activations instead of
   ones-matmuls into PSUM.
 - Encoder/head ReLU biases likewise.
 - Output bias b2 is added host-side after the gather.

This removes ~260 N=512 matmuls per core (~30% of tensor-engine time in
v2, which profiled at 85% busy).
"""

import numpy as np

import concourse.bass as bass
import concourse.bacc as bacc
import concourse.mybir as mybir
import concourse.tile as tile
from concourse import bass_utils

B, D_IN, D_GAS, D_OUT = 16384, 512, 256, 1000
N_PLATES = 8
N_CORES = 8
ROWS = B // N_CORES          # rows per core
N_CHUNKS = 4
R = ROWS // N_CHUNKS         # rows per chunk
F32 = mybir.dt.float32
BF16 = mybir.dt.bfloat16
NPBF16 = mybir.dt.np(BF16)
AF = mybir.ActivationFunctionType
OP = mybir.AluOpType


def _preprocess_weights(inp):
    """Fold the plate linear algebra host-side (float64, cast to bf16)."""
    f32, f64 = np.float32, np.float64
    W_tr = np.asarray(inp["W_tr"], f64)
    b_tr = np.asarray(inp["b_tr"], f64)
    W_ab = np.asarray(inp["W_ab"], f64)
    b_ab = np.asarray(inp["b_ab"], f64)
    W_eq = np.asarray(inp["W_eq"], f64)
    b_eq = np.asarray(inp["b_eq"], f64)
    W1 = np.asarray(inp["W1"], f64)
    b1 = np.asarray(inp["b1"], f64)
    alpha = float(np.asarray(inp["alpha"]))

    W_trp = alpha * W_tr
    ab_tr = alpha * b_tr
    W_trab = W_trp @ W_ab
    c2 = ab_tr @ W_ab + b_ab
    W_trabeq = W_trab @ W_eq
    c3 = c2 @ W_eq
    W1_g, W1_l = W1[:D_GAS], W1[D_GAS:]
    W_fold = W_trab @ W1_l
    e9 = 1.0 / (1.0 + np.exp(-b_eq))
    h_bias = b1 + 8.0 * (c2 @ W1_l)

    # ascending-bias deferral: stored g after n ascending plates is
    # g_true + delta_n with delta_n = delta_{n-1} @ (I - aW_tr) + a b_tr
    M = np.eye(D_GAS) - W_trp
    delta = [np.zeros(D_GAS)]
    for _ in range(N_PLATES):
        delta.append(delta[-1] @ M + ab_tr)
    # descending sigmoid bias, per sweep s and plate n:
    #   b_eq + (9-n) c3 - [s==1] * (sum_{m=n..8} delta_{m-1}) @ W_trabeq
    bias_tab = np.zeros((2, N_PLATES, D_GAS))
    for n in range(1, N_PLATES + 1):
        dsum = np.sum(delta[n - 1 : N_PLATES], axis=0)  # delta_{n-1}..delta_7
        bias_tab[0, n - 1] = b_eq + (9 - n) * c3
        bias_tab[1, n - 1] = b_eq + (9 - n) * c3 - dsum @ W_trabeq
    # head: g8 drifts by delta_8, S drifts by T = sum_{m=0..7} delta_m
    T = np.sum(delta[0:N_PLATES], axis=0)
    h_bias = h_bias - delta[N_PLATES] @ W1_g - T @ W_fold

    def kmaj(w):  # (K, M) -> [128, K//128, M] partition-major contraction
        w = np.asarray(w, NPBF16)
        k, m = w.shape
        return np.ascontiguousarray(w.reshape(k // 128, 128, m).transpose(1, 0, 2))

    def pscal(v):  # (256,) -> [128, 2] per-partition scalars (f32)
        return np.ascontiguousarray(np.asarray(v, f32).reshape(2, 128).T)

    return {
        "wge": kmaj(np.asarray(inp["W_ge"], f32)),
        "wdesc": kmaj(W_trabeq),
        "wasc": kmaj(-W_trp),
        "wfold": kmaj(W_fold),
        "w1g": kmaj(W1_g),
        "w2": kmaj(np.asarray(inp["W2"], f32)),
        "iden": np.eye(128, dtype=np.float32).astype(NPBF16),
        "bge": pscal(np.asarray(inp["b_ge"], f32)),
        "hb": pscal(h_bias),
        "ne9": pscal(-e9),
        # [128, sweep, plate, ft]
        "bes": np.ascontiguousarray(
            bias_tab.reshape(2, N_PLATES, 2, 128).transpose(3, 0, 1, 2).astype(f32)
        ),
    }


def build_nc():
    nc = bacc.Bacc("TRN2", target_bir_lowering=False, debug=False)

    x_d = nc.dram_tensor("x", (ROWS, D_IN), BF16, kind="ExternalInput").ap()
    wge_d = nc.dram_tensor("wge", (128, 4, D_GAS), BF16, kind="ExternalInput").ap()
    wdesc_d = nc.dram_tensor("wdesc", (128, 2, D_GAS), BF16, kind="ExternalInput").ap()
    wasc_d = nc.dram_tensor("wasc", (128, 2, D_GAS), BF16, kind="ExternalInput").ap()
    wfold_d = nc.dram_tensor("wfold", (128, 2, D_GAS), BF16, kind="ExternalInput").ap()
    w1g_d = nc.dram_tensor("w1g", (128, 2, D_GAS), BF16, kind="ExternalInput").ap()
    w2_d = nc.dram_tensor("w2", (128, 2, D_OUT), BF16, kind="ExternalInput").ap()
    iden_d = nc.dram_tensor("iden", (128, 128), BF16, kind="ExternalInput").ap()
    bge_d = nc.dram_tensor("bge", (128, 2), F32, kind="ExternalInput").ap()
    hb_d = nc.dram_tensor("hb", (128, 2), F32, kind="ExternalInput").ap()
    ne9_d = nc.dram_tensor("ne9", (128, 2), F32, kind="ExternalInput").ap()
    bes_d = nc.dram_tensor("bes", (128, 2, N_PLATES, 2), F32, kind="ExternalInput").ap()
    out_d = nc.dram_tensor("out", (ROWS, D_OUT), F32, kind="ExternalOutput").ap()

    with tile.TileContext(nc) as tc:
        with (
            tc.tile_pool(name="const", bufs=1) as cpool,
            tc.tile_pool(name="state", bufs=2) as spool,
            tc.tile_pool(name="work", bufs=3) as wpool,
            tc.tile_pool(name="psum", bufs=1, space="PSUM") as ppool,
        ):
            # ---- constants ----
            wge_t = cpool.tile([128, 4, D_GAS], BF16, tag="wge")
            nc.sync.dma_start(wge_t, wge_d)
            wdesc_t = cpool.tile([128, 2, D_GAS], BF16, tag="wdesc")
            nc.sync.dma_start(wdesc_t, wdesc_d)
            wasc_t = cpool.tile([128, 2, D_GAS], BF16, tag="wasc")
            nc.sync.dma_start(wasc_t, wasc_d)
            wfold_t = cpool.tile([128, 2, D_GAS], BF16, tag="wfold")
            nc.sync.dma_start(wfold_t, wfold_d)
            w1g_t = cpool.tile([128, 2, D_GAS], BF16, tag="w1g")
            nc.sync.dma_start(w1g_t, w1g_d)
            w2_t = cpool.tile([128, 2, D_OUT], BF16, tag="w2")
            nc.sync.dma_start(w2_t, w2_d)
            iden_t = cpool.tile([128, 128], BF16, tag="iden")
            nc.sync.dma_start(iden_t, iden_d)
            bge_t = cpool.tile([128, 2], F32, tag="bge")
            nc.sync.dma_start(bge_t, bge_d)
            hb_t = cpool.tile([128, 2], F32, tag="hb")
            nc.sync.dma_start(hb_t, hb_d)
            ne9_t = cpool.tile([128, 2], F32, tag="ne9")
            nc.sync.dma_start(ne9_t, ne9_d)
            bes_t = cpool.tile([128, 2, N_PLATES, 2], F32, tag="bes")
            nc.sync.dma_start(bes_t, bes_d)

            def wmm(acc, w_t, rhs, start=False, stop=False):
                """acc[128,2,R] += rhs @ W  (W given k-major [128,2,256])."""
                for ft in range(2):
                    for k in range(2):
                        nc.tensor.matmul(
                            acc[:, ft, :],
                            lhsT=w_t[:, k, ft * 128 : (ft + 1) * 128],
                            rhs=rhs[:, k, :],
                            start=start and k == 0,
                            stop=stop and k == 1,
                            skip_group_check=not (start and k == 0),
                        )

            def iden_mm(acc, rhs, start=False, stop=False):
                """acc[128,2,R] += rhs (injected through the PE identity)."""
                for ft in range(2):
                    nc.tensor.matmul(
                        acc[:, ft, :],
                        lhsT=iden_t,
                        rhs=rhs[:, ft, :],
                        start=start,
                        stop=stop,
                        skip_group_check=not start,
                    )

            g0s = {}

            def enc_gen(c):
                p = c % 2
                r0 = c * R

                # ---- x loaded transposed by the DMA xbar ----
                xT = spool.tile([128, 4, R], BF16, tag=f"xT{p}")
                nc.sync.dma_start_transpose(xT, x_d[r0 : r0 + R, :])
                yield

                # ---- encoder: g0 = relu(x @ W_ge + b_ge) ----
                enc = ppool.tile([128, 2, R], F32, tag="enc", name=f"enc{c}")
                for ft in range(2):
                    for k in range(4):
                        nc.tensor.matmul(
                            enc[:, ft, :],
                            lhsT=wge_t[:, k, ft * 128 : (ft + 1) * 128],
                            rhs=xT[:, k, :],
                            start=(k == 0),
                            stop=(k == 3),
                        )
                g0 = spool.tile([128, 2, R], BF16, tag=f"g0{p}")
                for ft in range(2):
                    nc.scalar.activation(
                        g0[:, ft, :], enc[:, ft, :], AF.Relu,
                        bias=bge_t[:, ft : ft + 1],
                    )
                g0s[c] = g0

            def chunk_gen(c):
                p = c % 2
                r0 = c * R
                g0 = g0s[c]

                acc = ppool.tile([128, 2, R], F32, tag=f"acc{p}", name=f"acc{c}")

                st = {0: g0}
                S = None
                for sweep in range(2):
                    last = sweep == 1
                    # ---------- descending sweep ----------
                    for n in range(N_PLATES, 0, -1):
                        df = wpool.tile([128, 2, R], BF16, tag=f"df{p}", bufs=3)
                        g_prev = st[n - 1] if (last and n > 1) else g0
                        # per-ft pipelining: df half k unblocks the k-matmuls
                        # while the other half's sigmoid still runs on ACT
                        for k in range(2):
                            if n == N_PLATES:
                                nc.vector.tensor_scalar(
                                    df[:, k, :], g_prev[:, k, :],
                                    ne9_t[:, k : k + 1], None, OP.add,
                                )
                            else:
                                nc.vector.tensor_tensor(
                                    df[:, k, :], g_prev[:, k, :],
                                    st[n + 1][:, k, :], OP.subtract,
                                )
                            for ft in range(2):
                                nc.tensor.matmul(
                                    acc[:, ft, :],
                                    lhsT=wdesc_t[:, k, ft * 128 : (ft + 1) * 128],
                                    rhs=df[:, k, :],
                                    start=(n == N_PLATES and k == 0),
                                    stop=(k == 1),
                                    skip_group_check=not (n == N_PLATES and k == 0),
                                )
                        if last:
                            if n == N_PLATES:
                                S = spool.tile([128, 2, R], BF16, tag=f"S{p}")
                                nc.gpsimd.tensor_copy(S, df)
                            else:
                                nc.gpsimd.tensor_tensor(S, S, df, OP.add)
                        e_new = spool.tile([128, 2, R], BF16, tag=f"st{p}_{n}")
                        for ft in range(2):
                            nc.scalar.activation(
                                e_new[:, ft, :], acc[:, ft, :], AF.Sigmoid,
                                bias=bes_t[:, sweep, n - 1, ft : ft + 1],
                            )
                        st[n] = e_new
                        yield

                    # ---------- ascending sweep ----------
                    if not last:
                        # materialize g_1..g_7 into SBUF for the next
                        # descending sweep (g_8 of sweep 0 is dead)
                        for n in range(1, N_PLATES):
                            df = wpool.tile([128, 2, R], BF16, tag=f"df{p}", bufs=3)
                            nc.vector.tensor_tensor(df, st[n - 1], st[n], OP.subtract)
                            use_act = n in (2, 4, 6)
                            wmm(acc, wasc_t, df, start=True, stop=not use_act)
                            g_new = spool.tile([128, 2, R], BF16, tag=f"st{p}_{n}")
                            if use_act:
                                iden_mm(acc, st[n - 1], stop=True)
                                nc.scalar.activation(g_new, acc, AF.Copy)
                            else:
                                nc.vector.tensor_tensor(g_new, st[n - 1], acc, OP.add)
                            st[n] = g_new
                            yield
                    else:
                        # keep g in the PSUM accumulator; only g_8 leaves
                        for n in range(1, N_PLATES + 1):
                            df = wpool.tile([128, 2, R], BF16, tag=f"df{p}", bufs=3)
                            if n == 1:
                                nc.vector.tensor_tensor(df, g0, st[1], OP.subtract)
                                iden_mm(acc, g0, start=True)
                                wmm(acc, wasc_t, df, stop=True)
                            else:
                                # both df halves must read acc BEFORE the
                                # matmuls mutate it (GPSIMD cannot read PSUM)
                                for k in range(2):
                                    nc.vector.tensor_tensor(
                                        df[:, k, :], acc[:, k, :],
                                        st[n][:, k, :], OP.subtract,
                                    )
                                wmm(acc, wasc_t, df, stop=True)
                            yield
                        g8 = spool.tile([128, 2, R], BF16, tag=f"st{p}_8")
                        nc.scalar.activation(g8, acc, AF.Copy)
                        st[N_PLATES] = g8
                        yield

                # ---------- head ----------
                wmm(acc, w1g_t, st[N_PLATES], start=True)
                wmm(acc, wfold_t, S, stop=True)
                h = spool.tile([128, 2, R], BF16, tag=f"h{p}")
                for ft in range(2):
                    nc.scalar.activation(
                        h[:, ft, :], acc[:, ft, :], AF.Relu,
                        bias=hb_t[:, ft : ft + 1],
                    )
                yield

                for rb in range(R // 128):
                    po = ppool.tile([128, D_OUT], F32, tag="po", bufs=1)
                    for n0, nw in ((0, 512), (512, 488)):
                        for ft in range(2):
                            nc.tensor.matmul(
                                po[:, n0 : n0 + nw],
                                lhsT=h[:, ft, rb * 128 : (rb + 1) * 128],
                                rhs=w2_t[:, ft, n0 : n0 + nw],
                                start=(ft == 0),
                                stop=(ft == 1),
                            )
                    stage = wpool.tile([128, D_OUT], F32, tag="stage", bufs=3)
                    nc.scalar.activation(stage, po, AF.Copy)
                    nc.sync.dma_start(
                        out_d[r0 + rb * 128 : r0 + (rb + 1) * 128, :], stage
                    )
                    yield

            # encoder phase for all chunks first (PE warm-up, removes the
            # pair-boundary bubble), then interleave each chunk pair
            def drive(gens):
                alive = list(gens)
                while alive:
                    for g in list(alive):
                        try:
                            next(g)
                        except StopIteration:
                            alive.remove(g)

            drive([enc_gen(c) for c in range(N_CHUNKS)])
            for pair in ((0, 1), (2, 3)):
                drive([chunk_gen(c) for c in pair])

    nc.compile()
    return nc


_NC_CACHE = {}


def _get_nc():
    if "nc" not in _NC_CACHE:
        _NC_CACHE["nc"] = build_nc()
    return _NC_CACHE["nc"]


def run_hw(inputs, trace=False):
    inp = {k: np.asarray(v) for k, v in inputs.items()}
    prep = _preprocess_weights(inp)
    x = np.asarray(inp["x"], dtype=np.float32).astype(NPBF16)
    b2 = np.asarray(inp["b2"], np.float32)

    nc = _get_nc()
    in_maps = []
    for c in range(N_CORES):
        m = {"x": np.ascontiguousarray(x[c * ROWS : (c + 1) * ROWS])}
        m.update(prep)
        in_maps.append(m)
    res = bass_utils.run_bass_kernel_spmd(
        nc, in_maps, core_ids=list(range(N_CORES)), trace=trace
    )
    out = np.concatenate([res.results[c]["out"] for c in range(N_CORES)], axis=0)
    out = out + b2.reshape(1, D_OUT)
    return out, res


def kernel(**inputs):
    out, _ = run_hw(inputs, trace=False)
    return out


# revision 17
# speedup vs baseline: 1.2336x; 1.2336x over previous
"""Trainium2 Bass kernel for nn_CounterFlowNetwork.

Data-parallel over 8 NeuronCores (batch sharded).  v3: on top of the v2
restructure (bf16 everywhere, DMA-xbar transposed x load, PSUM-resident
accumulators, chunk-pair issue interleaving), ALL K=1 bias-injection
matmuls are gone:

 - The per-plate ascending bias -alpha*b_tr is simply not applied on
   device.  The stored gas state drifts by a host-computable constant
   delta_n = delta_{n-1}(I - alpha W_tr) + alpha b_tr per ascending
   plate; the drift is corrected in the descending sigmoid bias table
   (per sweep/plate/ft), in the head bias (for g_8 and for S, the
   driving-force sum), all folded host-side in float64.
 - Descending sigmoid biases (b_eq + (9-n)c3 + drift correction) ride
   ACT's per-partition bias port with per-ft activations instead of
   ones-matmuls into PSUM.
 - Encoder/head ReLU biases likewise.
 - Output bias b2 is added host-side after the gather.

This removes ~260 N=512 matmuls per core (~30% of tensor-engine time in
v2, which profiled at 85% busy).
"""

import numpy as np

import concourse.bass as bass
import concourse.bacc as bacc
import concourse.mybir as mybir
import concourse.tile as tile
from concourse import bass_utils

B, D_IN, D_GAS, D_OUT = 16384, 512, 256, 1000
N_PLATES = 8
N_CORES = 8
ROWS = B // N_CORES          # rows per core
N_CHUNKS = 4
R = ROWS // N_CHUNKS         # rows per chunk
F32 = mybir.dt.float32
BF16 = mybir.dt.bfloat16
NPBF16 = mybir.dt.np(BF16)
AF = mybir.ActivationFunctionType
OP = mybir.AluOpType


def _preprocess_weights(inp):
    """Fold the plate linear algebra host-side (float64, cast to bf16)."""
    f32, f64 = np.float32, np.float64
    W_tr = np.asarray(inp["W_tr"], f64)
    b_tr = np.asarray(inp["b_tr"], f64)
    W_ab = np.asarray(inp["W_ab"], f64)
    b_ab = np.asarray(inp["b_ab"], f64)
    W_eq = np.asarray(inp["W_eq"], f64)
    b_eq = np.asarray(inp["b_eq"], f64)
    W1 = np.asarray(inp["W1"], f64)
    b1 = np.asarray(inp["b1"], f64)
    alpha = float(np.asarray(inp["alpha"]))

    W_trp = alpha * W_tr
    ab_tr = alpha * b_tr
    W_trab = W_trp @ W_ab
    c2 = ab_tr @ W_ab + b_ab
    W_trabeq = W_trab @ W_eq
    c3 = c2 @ W_eq
    W1_g, W1_l = W1[:D_GAS], W1[D_GAS:]
    W_fold = W_trab @ W1_l
    e9 = 1.0 / (1.0 + np.exp(-b_eq))
    h_bias = b1 + 8.0 * (c2 @ W1_l)

    # ascending-bias deferral: stored g after n ascending plates is
    # g_true + delta_n with delta_n = delta_{n-1} @ (I - aW_tr) + a b_tr
    M = np.eye(D_GAS) - W_trp
    delta = [np.zeros(D_GAS)]
    for _ in range(N_PLATES):
        delta.append(delta[-1] @ M + ab_tr)
    # descending sigmoid bias, per sweep s and plate n:
    #   b_eq + (9-n) c3 - [s==1] * (sum_{m=n..8} delta_{m-1}) @ W_trabeq
    # plate-8 df elimination: the matmul consumes g directly (df_8 = g - e9
    # with constant e9), so every pacc gains +e9 @ W_trabeq and the S sum
    # gains +e9 -- both folded into the biases below.
    bias_tab = np.zeros((2, N_PLATES, D_GAS))
    e9w = e9 @ W_trabeq
    for n in range(1, N_PLATES + 1):
        dsum = np.sum(delta[n - 1 : N_PLATES], axis=0)  # delta_{n-1}..delta_7
        bias_tab[0, n - 1] = b_eq + (9 - n) * c3 - e9w
        bias_tab[1, n - 1] = b_eq + (9 - n) * c3 - e9w - dsum @ W_trabeq
    # head: g8 drifts by delta_8, S drifts by T = sum_{m=0..7} delta_m (and
    # by +e9 from the plate-8 substitution)
    T = np.sum(delta[0:N_PLATES], axis=0)
    h_bias = h_bias - delta[N_PLATES] @ W1_g - (T + e9) @ W_fold

    def kmaj(w):  # (K, M) -> [128, K//128, M] partition-major contraction
        w = np.asarray(w, NPBF16)
        k, m = w.shape
        return np.ascontiguousarray(w.reshape(k // 128, 128, m).transpose(1, 0, 2))

    def pscal(v):  # (256,) -> [128, 2] per-partition scalars (f32)
        return np.ascontiguousarray(np.asarray(v, f32).reshape(2, 128).T)

    return {
        "wge": kmaj(np.asarray(inp["W_ge"], f32)),
        "wdesc": kmaj(W_trabeq),
        "wasc": kmaj(-W_trp),
        "wfold": kmaj(W_fold),
        "w1g": kmaj(W1_g),
        "w2": kmaj(np.asarray(inp["W2"], f32)),
        "iden": np.eye(128, dtype=np.float32).astype(NPBF16),
        "bge": pscal(np.asarray(inp["b_ge"], f32)),
        "hb": pscal(h_bias),
        # [128, sweep, plate, ft]
        "bes": np.ascontiguousarray(
            bias_tab.reshape(2, N_PLATES, 2, 128).transpose(3, 0, 1, 2).astype(f32)
        ),
    }


def build_nc():
    nc = bacc.Bacc("TRN2", target_bir_lowering=False, debug=False)

    x_d = nc.dram_tensor("x", (ROWS, D_IN), BF16, kind="ExternalInput").ap()
    wge_d = nc.dram_tensor("wge", (128, 4, D_GAS), BF16, kind="ExternalInput").ap()
    wdesc_d = nc.dram_tensor("wdesc", (128, 2, D_GAS), BF16, kind="ExternalInput").ap()
    wasc_d = nc.dram_tensor("wasc", (128, 2, D_GAS), BF16, kind="ExternalInput").ap()
    wfold_d = nc.dram_tensor("wfold", (128, 2, D_GAS), BF16, kind="ExternalInput").ap()
    w1g_d = nc.dram_tensor("w1g", (128, 2, D_GAS), BF16, kind="ExternalInput").ap()
    w2_d = nc.dram_tensor("w2", (128, 2, D_OUT), BF16, kind="ExternalInput").ap()
    iden_d = nc.dram_tensor("iden", (128, 128), BF16, kind="ExternalInput").ap()
    bge_d = nc.dram_tensor("bge", (128, 2), F32, kind="ExternalInput").ap()
    hb_d = nc.dram_tensor("hb", (128, 2), F32, kind="ExternalInput").ap()
    bes_d = nc.dram_tensor("bes", (128, 2, N_PLATES, 2), F32, kind="ExternalInput").ap()
    out_d = nc.dram_tensor("out", (ROWS, D_OUT), F32, kind="ExternalOutput").ap()

    with tile.TileContext(nc) as tc:
        with (
            tc.tile_pool(name="const", bufs=1) as cpool,
            tc.tile_pool(name="state", bufs=2) as spool,
            tc.tile_pool(name="work", bufs=3) as wpool,
            tc.tile_pool(name="psum", bufs=1, space="PSUM") as ppool,
        ):
            # ---- constants ----
            wge_t = cpool.tile([128, 4, D_GAS], BF16, tag="wge")
            nc.sync.dma_start(wge_t, wge_d)
            wdesc_t = cpool.tile([128, 2, D_GAS], BF16, tag="wdesc")
            nc.sync.dma_start(wdesc_t, wdesc_d)
            wasc_t = cpool.tile([128, 2, D_GAS], BF16, tag="wasc")
            nc.sync.dma_start(wasc_t, wasc_d)
            wfold_t = cpool.tile([128, 2, D_GAS], BF16, tag="wfold")
            nc.sync.dma_start(wfold_t, wfold_d)
            w1g_t = cpool.tile([128, 2, D_GAS], BF16, tag="w1g")
            nc.sync.dma_start(w1g_t, w1g_d)
            w2_t = cpool.tile([128, 2, D_OUT], BF16, tag="w2")
            nc.sync.dma_start(w2_t, w2_d)
            iden_t = cpool.tile([128, 128], BF16, tag="iden")
            nc.sync.dma_start(iden_t, iden_d)
            bge_t = cpool.tile([128, 2], F32, tag="bge")
            nc.sync.dma_start(bge_t, bge_d)
            hb_t = cpool.tile([128, 2], F32, tag="hb")
            nc.sync.dma_start(hb_t, hb_d)
            bes_t = cpool.tile([128, 2, N_PLATES, 2], F32, tag="bes")
            nc.sync.dma_start(bes_t, bes_d)

            def wmm(acc, w_t, rhs, start=False, stop=False):
                """acc[128,2,R] += rhs @ W  (W given k-major [128,2,256])."""
                for ft in range(2):
                    for k in range(2):
                        nc.tensor.matmul(
                            acc[:, ft, :],
                            lhsT=w_t[:, k, ft * 128 : (ft + 1) * 128],
                            rhs=rhs[:, k, :],
                            start=start and k == 0,
                            stop=stop and k == 1,
                            skip_group_check=not (start and k == 0),
                        )

            def iden_mm(acc, rhs, start=False, stop=False):
                """acc[128,2,R] += rhs (injected through the PE identity)."""
                for ft in range(2):
                    nc.tensor.matmul(
                        acc[:, ft, :],
                        lhsT=iden_t,
                        rhs=rhs[:, ft, :],
                        start=start,
                        stop=stop,
                        skip_group_check=not start,
                    )

            def chunk_gen(c):
                p = c % 2
                r0 = c * R

                # ---- x loaded transposed by the DMA xbar ----
                xT = spool.tile([128, 4, R], BF16, tag=f"xT{p}")
                nc.sync.dma_start_transpose(xT, x_d[r0 : r0 + R, :])

                acc = ppool.tile([128, 2, R], F32, tag=f"acc{p}", name=f"acc{c}")

                # ---- encoder: g0 = relu(x @ W_ge + b_ge) ----
                for ft in range(2):
                    for k in range(4):
                        nc.tensor.matmul(
                            acc[:, ft, :],
                            lhsT=wge_t[:, k, ft * 128 : (ft + 1) * 128],
                            rhs=xT[:, k, :],
                            start=(k == 0),
                            stop=(k == 3),
                        )
                g0 = spool.tile([128, 2, R], BF16, tag=f"g0{p}")
                for ft in range(2):
                    nc.scalar.activation(
                        g0[:, ft, :], acc[:, ft, :], AF.Relu,
                        bias=bge_t[:, ft : ft + 1],
                    )
                yield

                st = {0: g0}
                S = None
                for sweep in range(2):
                    last = sweep == 1
                    # ---------- descending sweep ----------
                    for n in range(N_PLATES, 0, -1):
                        g_prev = st[n - 1] if (last and n > 1) else g0
                        if n == N_PLATES:
                            # df_8 = g - e9 (const): feed g straight into the
                            # matmul; e9 terms are folded into the biases
                            rhs = g_prev
                            if last:
                                S = spool.tile([128, 2, R], BF16, tag=f"S{p}")
                                nc.gpsimd.tensor_copy(S, g_prev)
                        else:
                            df = wpool.tile([128, 2, R], BF16, tag=f"df{p}", bufs=3)
                            nc.vector.tensor_tensor(df, g_prev, st[n + 1], OP.subtract)
                            if last:
                                nc.gpsimd.tensor_tensor(S, S, df, OP.add)
                            rhs = df
                        wmm(acc, wdesc_t, rhs, start=(n == N_PLATES), stop=True)
                        e_new = spool.tile([128, 2, R], BF16, tag=f"st{p}_{n}")
                        for ft in range(2):
                            nc.scalar.activation(
                                e_new[:, ft, :], acc[:, ft, :], AF.Sigmoid,
                                bias=bes_t[:, sweep, n - 1, ft : ft + 1],
                            )
                        st[n] = e_new
                        yield

                    # ---------- ascending sweep ----------
                    if not last:
                        # materialize g_1..g_7 into SBUF for the next
                        # descending sweep (g_8 of sweep 0 is dead)
                        for n in range(1, N_PLATES):
                            df = wpool.tile([128, 2, R], BF16, tag=f"df{p}", bufs=3)
                            nc.vector.tensor_tensor(df, st[n - 1], st[n], OP.subtract)
                            wmm(acc, wasc_t, df, start=True, stop=True)
                            g_new = spool.tile([128, 2, R], BF16, tag=f"st{p}_{n}")
                            nc.vector.tensor_tensor(g_new, st[n - 1], acc, OP.add)
                            st[n] = g_new
                            yield
                    else:
                        # keep g in the PSUM accumulator; only g_8 leaves
                        for n in range(1, N_PLATES + 1):
                            df = wpool.tile([128, 2, R], BF16, tag=f"df{p}", bufs=3)
                            if n == 1:
                                nc.vector.tensor_tensor(df, g0, st[1], OP.subtract)
                                iden_mm(acc, g0, start=True)
                            else:
                                # GPSIMD cannot read PSUM; DVE does these
                                nc.vector.tensor_tensor(df, acc, st[n], OP.subtract)
                            wmm(acc, wasc_t, df, stop=True)
                            yield
                        g8 = spool.tile([128, 2, R], BF16, tag=f"st{p}_8")
                        nc.scalar.activation(g8, acc, AF.Copy)
                        st[N_PLATES] = g8
                        yield

                # ---------- head ----------
                wmm(acc, w1g_t, st[N_PLATES], start=True)
                wmm(acc, wfold_t, S, stop=True)
                h = spool.tile([128, 2, R], BF16, tag=f"h{p}")
                for ft in range(2):
                    nc.scalar.activation(
                        h[:, ft, :], acc[:, ft, :], AF.Relu,
                        bias=hb_t[:, ft : ft + 1],
                    )
                yield

                for rb in range(R // 128):
                    po = ppool.tile([128, D_OUT], F32, tag="po", bufs=2)
                    for n0, nw in ((0, 512), (512, 488)):
                        for ft in range(2):
                            nc.tensor.matmul(
                                po[:, n0 : n0 + nw],
                                lhsT=h[:, ft, rb * 128 : (rb + 1) * 128],
                                rhs=w2_t[:, ft, n0 : n0 + nw],
                                start=(ft == 0),
                                stop=(ft == 1),
                            )
                    stage = wpool.tile([128, D_OUT], F32, tag="stage", bufs=3)
                    nc.scalar.activation(stage, po, AF.Copy)
                    nc.sync.dma_start(
                        out_d[r0 + rb * 128 : r0 + (rb + 1) * 128, :], stage
                    )
                    yield

            # interleave issue order within each chunk pair
            for pair in ((0, 1), (2, 3)):
                gens = [chunk_gen(c) for c in pair]
                alive = list(gens)
                while alive:
                    for g in list(alive):
                        try:
                            next(g)
                        except StopIteration:
                            alive.remove(g)

    nc.compile()
    return nc


_NC_CACHE = {}


def _get_nc():
    if "nc" not in _NC_CACHE:
        _NC_CACHE["nc"] = build_nc()
    return _NC_CACHE["nc"]


def run_hw(inputs, trace=False):
    inp = {k: np.asarray(v) for k, v in inputs.items()}
    prep = _preprocess_weights(inp)
    x = np.asarray(inp["x"], dtype=np.float32).astype(NPBF16)
    b2 = np.asarray(inp["b2"], np.float32)

    nc = _get_nc()
    in_maps = []
    for c in range(N_CORES):
        m = {"x": np.ascontiguousarray(x[c * ROWS : (c + 1) * ROWS])}
        m.update(prep)
        in_maps.append(m)
    res = bass_utils.run_bass_kernel_spmd(
        nc, in_maps, core_ids=list(range(N_CORES)), trace=trace
    )
    out = np.concatenate([res.results[c]["out"] for c in range(N_CORES)], axis=0)
    out = out + b2.reshape(1, D_OUT)
    return out, res


def kernel(**inputs):
    out, _ = run_hw(inputs, trace=False)
    return out
